# revision 1
# baseline (speedup 1.0000x reference)
"""Trainium2 Bass kernel for nn_CausalLSTMNodeCell (B=1048576, D=32, H=16, C=3).

Strategy v2: pure data parallel over batch across 8 cores; each core's rows
split into three local sections s (rows b = m*R + s + 3t). Traffic reduction
vs v1: for child k the TF-reshape needs gate rows floor((k*B+b)/3), which is
the SAME row range (+-1 col shift) for all three sections, differing only in
which 16-col block of W_r applies. One gathered copy of [x|h|1] per child
serves all three sections, cutting xh traffic from 4 copies to 2.

Device structure is core-invariant: pair tile P_i carries section i's main
rows (partitions 0:49) and child k_i=(i+m)%3's gather rows (partitions 49:98),
chosen so phi=(k_i*B+m*R)%3 == i. Per triple j and i:
  pair matmul (K=98, N=128): psum block i cols 0:128 =
      [main gates of section i (96) | r-gates of child grp i, 2 "A" sections]
  B matmul (K=98 zero-top, N=16): block i cols 128:144 = remaining section.
Psum triple layout [3, 176] (last 32 pad); gates SBUF [SZ, 3, 144]. One
Sigmoid per psum group covers all gates (tanh'd "a" rides it via
2*sigmoid(2z)-1, weights pre-scaled). Elementwise on VectorE/GPSIMD; aux
(c_prev, child groups) bf16; outputs n,h,c bf16, upcast on host.
"""

import numpy as np
import ml_dtypes

B, D, H, C = 1048576, 32, 16, 3
NCORES = 8
R = B // NCORES            # 131072 rows per core
TPB = 342                  # triple-blocks per section (43776 >= 43691)
TP = TPB * 128
TPW = TP + 128             # +1 block for the delta col shifts
SZ = 18                    # triples per supergroup (342 = 19*18)
GRP = 3                    # triples per psum group
NSG = TPB // SZ            # 19
KDIM = 98
bf16 = ml_dtypes.bfloat16

# core-invariant plan: phi=i => delta pattern over sections
DA = (0, 0, 1)             # pair-matmul col shift per block i
DB = (0, 1, 0)             # B-matmul col shift per block i
SA = ((0, 1), (0, 1), (1, 2))   # the two "A" sections of block i
SB = (2, 2, 0)                  # the "B" section of block i
# r-gate col within block i for section s (96 + 16*pos, B at 128)
RCOL = [[96, 96, 128], [112, 112, 96], [128, 128, 112]]  # [s][i]

XT_BUFS = 3
AXD_BUFS = 3
SG_BUFS = 3
GATES_BF16 = True          # gates tile bf16 (DVE 2x mode)
TMP_BF16 = True            # tmp tile bf16 (DVE 2x mode)
GPS_RMUL2 = False          # rmul block-2 trio on GPSIMD
GPS_N2H = False            # n2h on GPSIMD
GPS_FC = False             # f*c_prev on GPSIMD
STORE_GPS = True           # res store trigger on Pool queue (not ACT)
AUX_ACT = True             # aux load trigger on ACT HWDGE ring (parallel to SP)
SPLIT_P = 0                # 0: P all on SP; 1: P[2] on DVE; 2: P[2] on ACT
LAG = 1
# timing-only ablation flags (break numerics; for bottleneck attribution)
ABL_NO_B = False           # skip the two B matmuls per triple
ABL_NO_TANH = False        # skip tanh
ABL_DVE_LITE = False       # skip the r-combination DVE ops
ABL_NO_AUX = False         # skip aux loads
ABL_NO_STORE = False       # skip res stores

_NC_CACHE = {}


def _build_w(inputs):
    """W [49, 144]: rows 0:32 x-w, 32:48 h-w, 48 bias. Cols: ifo 0:48,
    n1 48:64, n2 64:80, a(x2) 80:96, r 96:144."""
    W = np.zeros((49, 144), np.float32)

    def put(cols, wx, wh, bx, bh, scale=1.0):
        W[0:32, cols] = scale * np.asarray(inputs[wx], np.float32)
        W[32:48, cols] = scale * np.asarray(inputs[wh], np.float32)
        W[48, cols] = scale * (np.asarray(inputs[bx], np.float32)
                               + np.asarray(inputs[bh], np.float32))

    put(slice(0, 48), "W_ifo_x", "W_ifo_h", "b_ifo_x", "b_ifo_h")
    put(slice(48, 64), "W_n1_x", "W_n1_h", "b_n1_x", "b_n1_h")
    put(slice(64, 80), "W_n2_x", "W_n2_h", "b_n2_x", "b_n2_h")
    put(slice(80, 96), "W_a_x", "W_a_h", "b_a_x", "b_a_h", scale=2.0)
    put(slice(96, 144), "W_r_x", "W_r_h", "b_r_x", "b_r_h")
    return W


def host_prep(inputs):
    x = np.asarray(inputs["inputs"], np.float32)
    hp = np.asarray(inputs["h_prev"], np.float32)
    cp = np.asarray(inputs["c_prev"], np.float32)
    ch = np.asarray(inputs["child_n"], np.float32)
    W = _build_w(inputs)
    Wr = W[:, 96:144]
    xh = np.zeros((B + 1, 49), np.float32)
    xh[:B, 0:32] = x
    xh[:B, 32:48] = hp
    xh[:B, 48] = 1.0
    xh16 = xh.astype(bf16)

    cp16 = np.concatenate([cp.astype(bf16), np.zeros((1, 16), bf16)])
    ch16 = [np.concatenate([ch[k].astype(bf16), np.zeros((1, 16), bf16)])
            for k in range(3)]

    # stationary weights: core-invariant; i=0's B cols fold into its pair
    # stationary (delta_A == delta_B for block 0), so wp is [3, KDIM, 144]
    wp = np.zeros((3, KDIM, 144), np.float32)
    wb = np.zeros((3, KDIM, 16), np.float32)
    for i in range(3):
        wp[i, 0:49, 0:96] = W[:, 0:96]
        ca0 = 16 * ((i + SA[i][0]) % 3)
        ca1 = 16 * ((i + SA[i][1]) % 3)
        cb = 16 * ((i + SB[i]) % 3)
        wp[i, 49:98, 96:112] = Wr[:, ca0:ca0 + 16]
        wp[i, 49:98, 112:128] = Wr[:, ca1:ca1 + 16]
        if i == 0:
            wp[i, 49:98, 128:144] = Wr[:, cb:cb + 16]
        else:
            wb[i, 49:98, 0:16] = Wr[:, cb:cb + 16]
    wp16, wb16 = wp.astype(bf16), wb.astype(bf16)

    cores = []
    u = np.arange(TPW)
    t_tp = np.arange(TP)
    for m in range(NCORES):
        pt = np.zeros((3, KDIM, TPW), bf16)
        aux = np.empty((TPB, 3, 128, 64), bf16)
        for i in range(3):
            first = m * R + i
            T = len(range(first, (m + 1) * R, 3))
            k = (i + m) % 3
            qmin = (k * B + m * R) // 3
            # main half: col u = xh[first + 3*(u - DA[i])]
            t = u - DA[i]
            bidx = np.where((t < 0) | (t >= T), B, first + 3 * t)
            pt[i, 0:49, :] = xh16[bidx].T
            # child half: col u = xh[qmin + u]
            gidx = np.minimum(qmin + u, B)
            pt[i, 49:98, :] = xh16[gidx].T
            # aux for section i: [cp | ch_grp0 | ch_grp1 | ch_grp2]
            bidx2 = np.where(t_tp < T, first + 3 * t_tp, B)
            sec = np.empty((TP, 64), bf16)
            sec[:, 0:16] = cp16[bidx2]
            for g in range(3):
                sec[:, 16 + 16 * g:32 + 16 * g] = ch16[(g + m) % 3][bidx2]
            aux[:, i, :, :] = sec.reshape(TPB, 128, 64)
        auxt = np.ascontiguousarray(aux.transpose(2, 0, 1, 3))  # [128,TPB,3,64]
        pall = np.ascontiguousarray(pt.transpose(1, 0, 2))  # [KDIM, 3, TPW]
        cores.append(dict(pall=pall, wp=wp16, wb=wb16, aux=auxt))
    return cores


def build_nc(niter=1):
    import concourse.tile as tile
    from concourse import bacc, mybir

    f32 = mybir.dt.float32
    b16 = mybir.dt.bfloat16
    AF = mybir.ActivationFunctionType
    ALU = mybir.AluOpType

    nc = bacc.Bacc(None, target_bir_lowering=False)
    p_d = nc.dram_tensor("pall", [KDIM, 3, TPW], b16, kind="ExternalInput")
    wp_d = nc.dram_tensor("wp", [3, KDIM, 144], b16, kind="ExternalInput")
    wb_d = nc.dram_tensor("wb", [3, KDIM, 16], b16, kind="ExternalInput")
    aux_d = nc.dram_tensor("aux", [128, TPB, 3, 64], b16, kind="ExternalInput")
    res_d = nc.dram_tensor("res", [128, TPB, 3, 48], b16,
                           kind="ExternalOutput")

    # tmp scratch slices (f32), [128, SZ, 3, 96]
    S0, S1, S2, S3, S4, S5 = (slice(16 * i, 16 * i + 16) for i in range(6))
    # gate cols within a 144-block
    GI, GF, GO = slice(0, 16), slice(16, 32), slice(32, 48)
    GN1, GN2, GA = slice(48, 64), slice(64, 80), slice(80, 96)

    with tile.TileContext(nc) as tc:
        with (
            tc.tile_pool(name="wpool", bufs=1) as wpool,
            tc.tile_pool(name="xtab", bufs=XT_BUFS) as xtabp,
            tc.tile_pool(name="axd", bufs=AXD_BUFS) as axdp,
            tc.tile_pool(name="gates", bufs=SG_BUFS) as gatesp,
            tc.tile_pool(name="tmp", bufs=SG_BUFS) as tmpp,
            tc.tile_pool(name="res", bufs=SG_BUFS) as resp,
            tc.tile_pool(name="ps", bufs=2, space="PSUM") as psp,
        ):
            wp_t = wpool.tile([KDIM, 3, 144], b16, tag="wp")
            wb_t = wpool.tile([KDIM, 3, 16], b16, tag="wb")
            for i in range(3):
                nc.sync.dma_start(wp_t[:, i, :], wp_d[i])
                nc.sync.dma_start(wb_t[:, i, :], wb_d[i])

            V = nc.vector
            G = nc.gpsimd
            E_RM2 = G if GPS_RMUL2 else V
            E_N2H = G if GPS_N2H else V
            E_FC = G if GPS_FC else V

            def gate_phase(sg):
                blk0 = sg * SZ
                sz = min(SZ, TPB - blk0)
                c0 = blk0 * 128
                cw = sz * 128 + 128
                pt = xtabp.tile([KDIM, 3, cw], b16, tag="pall")
                if SPLIT_P == 0:
                    nc.sync.dma_start(pt[:], p_d[:, :, c0:c0 + cw])
                else:
                    PE2 = nc.gpsimd if SPLIT_P == 1 else nc.scalar
                    nc.sync.dma_start(pt[:, 0:2, :], p_d[:, 0:2, c0:c0 + cw])
                    PE2.dma_start(pt[:, 2, :], p_d[:, 2, c0:c0 + cw])
                auxt = axdp.tile([128, sz, 3, 64], b16, tag="aux")
                AUXE = nc.scalar if AUX_ACT else nc.sync
                if ABL_NO_AUX:
                    # tiny load keeps the tile written; kills the traffic
                    AUXE.dma_start(auxt[:, 0:1, :, :],
                                   aux_d[:, blk0:blk0 + 1, :, :])
                else:
                    AUXE.dma_start(auxt[:], aux_d[:, blk0:blk0 + sz, :, :])

                gates = gatesp.tile([128, sz, 3, 144],
                                    b16 if GATES_BF16 else f32, tag="gates")
                tmp = tmpp.tile([128, sz, 3, 96],
                                b16 if TMP_BF16 else f32, tag="tmp")
                res = resp.tile([128, sz, 3, 48], b16, tag="res")
                for g in range(-(-sz // GRP)):
                    gsz = min(GRP, sz - g * GRP)
                    ps = psp.tile([128, gsz, 3, 176], f32, tag="ps")
                    for jj in range(gsz):
                        j = g * GRP + jj
                        for i in range(3):
                            nw = 144 if i == 0 else 128
                            nc.tensor.matmul(
                                ps[:, jj, i, 0:nw],
                                pt[:, i, j * 128 + DA[i]:
                                   j * 128 + DA[i] + 128],
                                wp_t[:, i, 0:nw])
                            if i != 0 and not ABL_NO_B:
                                nc.tensor.matmul(
                                    ps[:, jj, i, 128:144],
                                    pt[:, i, j * 128 + DB[i]:
                                       j * 128 + DB[i] + 128],
                                    wb_t[:, i, :])
                    gsl = slice(g * GRP, g * GRP + gsz)
                    nc.scalar.activation(
                        gates[:, gsl, :, :], ps[:, :, :, 0:144], AF.Sigmoid)
                return (gates, auxt, tmp, res, blk0, sz)

            def elem_phase(state):
                gates, auxt, tmp, res, blk0, sz = state
                ALL = slice(None)
                g3 = (ALL, ALL, ALL)
                # a = 2*sigmoid(2z) - 1
                V.tensor_scalar(tmp[:, :, :, S5], gates[:, :, :, GA],
                                2.0, -1.0, ALU.mult, ALU.add)
                V.tensor_mul(tmp[:, :, :, S0], gates[:, :, :, GI],
                             tmp[:, :, :, S5])
                E_FC.tensor_mul(tmp[:, :, :, S1], gates[:, :, :, GF],
                                auxt[:, :, :, 0:16])
                V.tensor_add(res[:, :, :, 32:48], tmp[:, :, :, S0],
                             tmp[:, :, :, S1])
                if not ABL_NO_TANH:
                    nc.scalar.activation(tmp[:, :, :, S2],
                                         res[:, :, :, 32:48], AF.Tanh)
                V.tensor_mul(res[:, :, :, 16:32], gates[:, :, :, GO],
                             tmp[:, :, :, S2])
                # r-gate * child products: blocks 0,1 are s-ordered 96:144
                if not ABL_DVE_LITE:
                    V.tensor_mul(tmp[:, :, :, S3], gates[:, :, 0, 96:144],
                                 auxt[:, :, :, 16:32])
                    V.tensor_mul(tmp[:, :, :, S4], gates[:, :, 1, 96:144],
                                 auxt[:, :, :, 32:48])
                    for s in range(3):
                        E_RM2.tensor_mul(
                            tmp[:, :, s, S0], gates[:, :, 2, RCOL[s][2]:
                                                    RCOL[s][2] + 16],
                            auxt[:, :, s, 48:64])
                    V.tensor_add(tmp[:, :, :, S5], tmp[:, :, :, S3],
                                 tmp[:, :, :, S4])
                    V.tensor_add(tmp[:, :, :, S3], tmp[:, :, :, S5],
                                 tmp[:, :, :, S0])
                V.tensor_mul(tmp[:, :, :, S4], gates[:, :, :, GN1],
                             tmp[:, :, :, S3])
                E_N2H.tensor_mul(tmp[:, :, :, S0], gates[:, :, :, GN2],
                                 res[:, :, :, 16:32])
                V.tensor_add(res[:, :, :, 0:16], tmp[:, :, :, S4],
                             tmp[:, :, :, S0])
                # stores ride the Pool queue (loads use SP) to keep the ACT
                # engine free for sigmoids and avoid head-of-line blocking
                if ABL_NO_STORE:
                    pass
                elif STORE_GPS:
                    nc.gpsimd.dma_start(res_d[:, blk0:blk0 + sz, :, :], res[:])
                else:
                    nc.scalar.dma_start(res_d[:, blk0:blk0 + sz, :, :], res[:])

            def one_pass():
                state = gate_phase(0)
                for sg in range(1, NSG):
                    nstate = gate_phase(sg)
                    elem_phase(state)
                    state = nstate
                elem_phase(state)

            if niter == 1:
                one_pass()
            else:
                with tc.For_i(0, niter):
                    one_pass()

    nc.compile()
    return nc


def _get_nc():
    if "nc" not in _NC_CACHE:
        _NC_CACHE["nc"] = build_nc()
    return _NC_CACHE["nc"]


def gather_out(results):
    n = np.empty((B, 16), np.float32)
    h = np.empty((B, 16), np.float32)
    c = np.empty((B, 16), np.float32)
    for m in range(NCORES):
        res = np.asarray(results[m]["res"]).astype(np.float32)
        # [128, TPB, 3, 48] -> per section s: [TP, 48]
        for s in range(3):
            first = m * R + s
            T = len(range(first, (m + 1) * R, 3))
            flat = res[:, :, s, :].transpose(1, 0, 2).reshape(TP, 48)
            n[first:(m + 1) * R:3] = flat[:T, 0:16]
            h[first:(m + 1) * R:3] = flat[:T, 16:32]
            c[first:(m + 1) * R:3] = flat[:T, 32:48]
    return n, h, c


def make_in_maps(cores):
    return [dict(pall=c["pall"], wp=c["wp"], wb=c["wb"], aux=c["aux"])
            for c in cores]


def kernel(**inputs):
    from concourse.bass_utils import run_bass_kernel_spmd

    cores = host_prep(inputs)
    nc = _get_nc()
    out = run_bass_kernel_spmd(nc, make_in_maps(cores),
                               core_ids=list(range(NCORES)))
    return gather_out(out.results)



# revision 20
# speedup vs baseline: 19.2458x; 19.2458x over previous
"""Trainium2 Bass kernel for nn_CausalLSTMNodeCell (B=1048576, D=32, H=16, C=3).

Strategy v2: pure data parallel over batch across 8 cores; each core's rows
split into three local sections s (rows b = m*R + s + 3t). Traffic reduction
vs v1: for child k the TF-reshape needs gate rows floor((k*B+b)/3), which is
the SAME row range (+-1 col shift) for all three sections, differing only in
which 16-col block of W_r applies. One gathered copy of [x|h|1] per child
serves all three sections, cutting xh traffic from 4 copies to 2.

Device structure is core-invariant: pair tile P_i carries section i's main
rows (partitions 0:49) and child k_i=(i+m)%3's gather rows (partitions 49:98),
chosen so phi=(k_i*B+m*R)%3 == i. Per triple j and i:
  pair matmul (K=98, N=128): psum block i cols 0:128 =
      [main gates of section i (96) | r-gates of child grp i, 2 "A" sections]
  B matmul (K=98 zero-top, N=16): block i cols 128:144 = remaining section.
Psum triple layout [3, 176] (last 32 pad); gates SBUF [SZ, 3, 144]. One
Sigmoid per psum group covers all gates (tanh'd "a" rides it via
2*sigmoid(2z)-1, weights pre-scaled). Elementwise on VectorE/GPSIMD; aux
(c_prev, child groups) bf16; outputs n,h,c bf16, upcast on host.
"""

import numpy as np
import ml_dtypes

B, D, H, C = 1048576, 32, 16, 3
NCORES = 8
R = B // NCORES            # 131072 rows per core
TPB = 342                  # triple-blocks per section (43776 >= 43691)
TP = TPB * 128
TPW = TP + 128             # +1 block for the delta col shifts
SZ = 18                    # triples per supergroup (342 = 19*18)
GRP = 3                    # triples per psum group
NSG = TPB // SZ            # 19
KDIM = 98
bf16 = ml_dtypes.bfloat16

# core-invariant plan: phi=i => delta pattern over sections
DA = (0, 0, 1)             # pair-matmul col shift per block i
DB = (0, 1, 0)             # B-matmul col shift per block i
SA = ((0, 1), (0, 1), (1, 2))   # the two "A" sections of block i
SB = (2, 2, 0)                  # the "B" section of block i
# r-gate col within block i for section s (96 + 16*pos, B at 128)
RCOL = [[96, 96, 128], [112, 112, 96], [128, 128, 112]]  # [s][i]

XT_BUFS = 3
AXD_BUFS = 3
SG_BUFS = 3
GATES_BF16 = True          # gates tile bf16 (DVE 2x mode)
TMP_BF16 = True            # tmp tile bf16 (DVE 2x mode)
GPS_RMUL2 = False          # rmul block-2 trio on GPSIMD
GPS_N2H = False            # n2h on GPSIMD
GPS_FC = False             # f*c_prev on GPSIMD
STORE_GPS = True           # res store trigger on Pool queue (not ACT)
AUX_ACT = True             # aux load trigger on ACT HWDGE ring (parallel to SP)
SPLIT_P = 0                # 0: P all on SP; 1: P[2] on DVE; 2: P[2] on ACT
LAG = 1
# timing-only ablation flags (break numerics; for bottleneck attribution)
ABL_NO_B = False           # skip the two B matmuls per triple
ABL_NO_TANH = False        # skip tanh
ABL_DVE_LITE = False       # skip the r-combination DVE ops
ABL_NO_AUX = False         # skip aux loads
ABL_NO_STORE = False       # skip res stores
ABL_HALF_SIG = False       # sigmoid only cols 0:72 (halves ACT stream)
ABL_NO_PALL = False        # shrink pall load to 1 block (kills xh traffic)
STAGGER = False            # staggered_reset on the niter loop
HINTS = False              # hint_engines on the niter loop
TRIM_TAIL = False          # skip the +128 col tail for section 0 (no shift)
MM_REORDER = True          # i-major matmul order (stationary reuse per group)
SG_MAJOR = True            # supergroup-major DRAM layout: each sg load/store
                           # is one fully-contiguous DRAM block
CW = SZ * 128 + 128        # per-sg pall cols (incl +1 block tail)
PALL_CHUNKS = 3            # split each sg pall load into N chunks (SG_MAJOR)
AUX_CHUNKS = 1             # split each sg aux load into N chunks (SG_MAJOR)

_NC_CACHE = {}


def _build_w(inputs):
    """W [49, 144]: rows 0:32 x-w, 32:48 h-w, 48 bias. Cols: ifo 0:48,
    n1 48:64, n2 64:80, a(x2) 80:96, r 96:144."""
    W = np.zeros((49, 144), np.float32)

    def put(cols, wx, wh, bx, bh, scale=1.0):
        W[0:32, cols] = scale * np.asarray(inputs[wx], np.float32)
        W[32:48, cols] = scale * np.asarray(inputs[wh], np.float32)
        W[48, cols] = scale * (np.asarray(inputs[bx], np.float32)
                               + np.asarray(inputs[bh], np.float32))

    put(slice(0, 48), "W_ifo_x", "W_ifo_h", "b_ifo_x", "b_ifo_h")
    put(slice(48, 64), "W_n1_x", "W_n1_h", "b_n1_x", "b_n1_h")
    put(slice(64, 80), "W_n2_x", "W_n2_h", "b_n2_x", "b_n2_h")
    put(slice(80, 96), "W_a_x", "W_a_h", "b_a_x", "b_a_h", scale=2.0)
    put(slice(96, 144), "W_r_x", "W_r_h", "b_r_x", "b_r_h")
    return W


def host_prep(inputs):
    x = np.asarray(inputs["inputs"], np.float32)
    hp = np.asarray(inputs["h_prev"], np.float32)
    cp = np.asarray(inputs["c_prev"], np.float32)
    ch = np.asarray(inputs["child_n"], np.float32)
    W = _build_w(inputs)
    Wr = W[:, 96:144]
    xh = np.zeros((B + 1, 49), np.float32)
    xh[:B, 0:32] = x
    xh[:B, 32:48] = hp
    xh[:B, 48] = 1.0
    xh16 = xh.astype(bf16)

    cp16 = np.concatenate([cp.astype(bf16), np.zeros((1, 16), bf16)])
    ch16 = [np.concatenate([ch[k].astype(bf16), np.zeros((1, 16), bf16)])
            for k in range(3)]

    # stationary weights: core-invariant; i=0's B cols fold into its pair
    # stationary (delta_A == delta_B for block 0), so wp is [3, KDIM, 144]
    wp = np.zeros((3, KDIM, 144), np.float32)
    wb = np.zeros((3, KDIM, 16), np.float32)
    for i in range(3):
        wp[i, 0:49, 0:96] = W[:, 0:96]
        ca0 = 16 * ((i + SA[i][0]) % 3)
        ca1 = 16 * ((i + SA[i][1]) % 3)
        cb = 16 * ((i + SB[i]) % 3)
        wp[i, 49:98, 96:112] = Wr[:, ca0:ca0 + 16]
        wp[i, 49:98, 112:128] = Wr[:, ca1:ca1 + 16]
        if i == 0:
            wp[i, 49:98, 128:144] = Wr[:, cb:cb + 16]
        else:
            wb[i, 49:98, 0:16] = Wr[:, cb:cb + 16]
    wp16, wb16 = wp.astype(bf16), wb.astype(bf16)

    cores = []
    u = np.arange(TPW)
    t_tp = np.arange(TP)
    for m in range(NCORES):
        pt = np.zeros((3, KDIM, TPW), bf16)
        aux = np.empty((TPB, 3, 128, 64), bf16)
        for i in range(3):
            first = m * R + i
            T = len(range(first, (m + 1) * R, 3))
            k = (i + m) % 3
            qmin = (k * B + m * R) // 3
            # main half: col u = xh[first + 3*(u - DA[i])]
            t = u - DA[i]
            bidx = np.where((t < 0) | (t >= T), B, first + 3 * t)
            pt[i, 0:49, :] = xh16[bidx].T
            # child half: col u = xh[qmin + u]
            gidx = np.minimum(qmin + u, B)
            pt[i, 49:98, :] = xh16[gidx].T
            # aux for section i: [cp | ch_grp0 | ch_grp1 | ch_grp2]
            bidx2 = np.where(t_tp < T, first + 3 * t_tp, B)
            sec = np.empty((TP, 64), bf16)
            sec[:, 0:16] = cp16[bidx2]
            for g in range(3):
                sec[:, 16 + 16 * g:32 + 16 * g] = ch16[(g + m) % 3][bidx2]
            aux[:, i, :, :] = sec.reshape(TPB, 128, 64)
        auxt = np.ascontiguousarray(aux.transpose(2, 0, 1, 3))  # [128,TPB,3,64]
        pall = np.ascontiguousarray(pt.transpose(1, 0, 2))  # [KDIM, 3, TPW]
        if SG_MAJOR:
            # [NSG, KDIM, 3, CW]: per-sg pall block contiguous in DRAM
            pall = np.ascontiguousarray(np.stack(
                [pall[:, :, sg * SZ * 128:sg * SZ * 128 + CW]
                 for sg in range(NSG)]))
            # [NSG, 128, SZ, 3, 64]: per-sg aux block contiguous
            auxt = np.ascontiguousarray(
                auxt.reshape(128, NSG, SZ, 3, 64).transpose(1, 0, 2, 3, 4))
        cores.append(dict(pall=pall, wp=wp16, wb=wb16, aux=auxt))
    return cores


def build_nc(niter=1):
    import concourse.tile as tile
    from concourse import bacc, mybir

    f32 = mybir.dt.float32
    b16 = mybir.dt.bfloat16
    AF = mybir.ActivationFunctionType
    ALU = mybir.AluOpType

    nc = bacc.Bacc(None, target_bir_lowering=False)
    if SG_MAJOR:
        p_d = nc.dram_tensor("pall", [NSG, KDIM, 3, CW], b16,
                             kind="ExternalInput")
        aux_d = nc.dram_tensor("aux", [NSG, 128, SZ, 3, 64], b16,
                               kind="ExternalInput")
        res_d = nc.dram_tensor("res", [NSG, 128, SZ, 3, 48], b16,
                               kind="ExternalOutput")
    else:
        p_d = nc.dram_tensor("pall", [KDIM, 3, TPW], b16,
                             kind="ExternalInput")
        aux_d = nc.dram_tensor("aux", [128, TPB, 3, 64], b16,
                               kind="ExternalInput")
        res_d = nc.dram_tensor("res", [128, TPB, 3, 48], b16,
                               kind="ExternalOutput")
    wp_d = nc.dram_tensor("wp", [3, KDIM, 144], b16, kind="ExternalInput")
    wb_d = nc.dram_tensor("wb", [3, KDIM, 16], b16, kind="ExternalInput")

    # tmp scratch slices (f32), [128, SZ, 3, 96]
    S0, S1, S2, S3, S4, S5 = (slice(16 * i, 16 * i + 16) for i in range(6))
    # gate cols within a 144-block
    GI, GF, GO = slice(0, 16), slice(16, 32), slice(32, 48)
    GN1, GN2, GA = slice(48, 64), slice(64, 80), slice(80, 96)

    with tile.TileContext(nc) as tc:
        with (
            tc.tile_pool(name="wpool", bufs=1) as wpool,
            tc.tile_pool(name="xtab", bufs=XT_BUFS) as xtabp,
            tc.tile_pool(name="axd", bufs=AXD_BUFS) as axdp,
            tc.tile_pool(name="gates", bufs=SG_BUFS) as gatesp,
            tc.tile_pool(name="tmp", bufs=SG_BUFS) as tmpp,
            tc.tile_pool(name="res", bufs=SG_BUFS) as resp,
            tc.tile_pool(name="ps", bufs=2, space="PSUM") as psp,
        ):
            wp_t = wpool.tile([KDIM, 3, 144], b16, tag="wp")
            wb_t = wpool.tile([KDIM, 3, 16], b16, tag="wb")
            for i in range(3):
                nc.sync.dma_start(wp_t[:, i, :], wp_d[i])
                nc.sync.dma_start(wb_t[:, i, :], wb_d[i])

            V = nc.vector
            G = nc.gpsimd
            E_RM2 = G if GPS_RMUL2 else V
            E_N2H = G if GPS_N2H else V
            E_FC = G if GPS_FC else V

            def gate_phase(sg):
                blk0 = sg * SZ
                sz = min(SZ, TPB - blk0)
                c0 = blk0 * 128
                cw = sz * 128 + 128
                pt = xtabp.tile([KDIM, 3, cw], b16, tag="pall")
                if ABL_NO_PALL:
                    src = p_d[sg, :, :, 0:128] if SG_MAJOR \
                        else p_d[:, :, c0:c0 + 128]
                    nc.sync.dma_start(pt[:, :, 0:128], src)
                elif SG_MAJOR:
                    if PALL_CHUNKS == 1:
                        nc.sync.dma_start(pt[:], p_d[sg])
                    else:
                        # chunk along the col dim at GRP-triple boundaries
                        step = (sz // PALL_CHUNKS) * 128
                        edges = list(range(0, sz * 128, step)) + [cw]
                        for a, b in zip(edges[:-1], edges[1:]):
                            nc.sync.dma_start(pt[:, :, a:b],
                                              p_d[sg, :, :, a:b])
                elif TRIM_TAIL and SPLIT_P == 0:
                    nc.sync.dma_start(pt[:, 0, 0:cw - 128],
                                      p_d[:, 0, c0:c0 + cw - 128])
                    nc.sync.dma_start(pt[:, 1:3, :], p_d[:, 1:3, c0:c0 + cw])
                elif SPLIT_P == 0:
                    nc.sync.dma_start(pt[:], p_d[:, :, c0:c0 + cw])
                else:
                    PE2 = nc.gpsimd if SPLIT_P == 1 else nc.scalar
                    nc.sync.dma_start(pt[:, 0:2, :], p_d[:, 0:2, c0:c0 + cw])
                    PE2.dma_start(pt[:, 2, :], p_d[:, 2, c0:c0 + cw])
                auxt = axdp.tile([128, sz, 3, 64], b16, tag="aux")
                AUXE = nc.scalar if AUX_ACT else nc.sync
                aux_src = aux_d[sg] if SG_MAJOR \
                    else aux_d[:, blk0:blk0 + sz, :, :]
                if ABL_NO_AUX:
                    # tiny load keeps the tile written; kills the traffic
                    AUXE.dma_start(auxt[:, 0:1, :, :],
                                   aux_src[:, 0:1] if SG_MAJOR
                                   else aux_d[:, blk0:blk0 + 1, :, :])
                elif SG_MAJOR and AUX_CHUNKS > 1:
                    step = sz // AUX_CHUNKS
                    for a in range(0, sz, step):
                        b = min(a + step, sz)
                        AUXE.dma_start(auxt[:, a:b], aux_src[:, a:b])
                else:
                    AUXE.dma_start(auxt[:], aux_src)

                gates = gatesp.tile([128, sz, 3, 144],
                                    b16 if GATES_BF16 else f32, tag="gates")
                tmp = tmpp.tile([128, sz, 3, 96],
                                b16 if TMP_BF16 else f32, tag="tmp")
                res = resp.tile([128, sz, 3, 48], b16, tag="res")
                for g in range(-(-sz // GRP)):
                    gsz = min(GRP, sz - g * GRP)
                    ps = psp.tile([128, gsz, 3, 176], f32, tag="ps")
                    if MM_REORDER:
                        # i-major: one stationary serves gsz matmuls
                        for i in range(3):
                            nw = 144 if i == 0 else 128
                            for jj in range(gsz):
                                j = g * GRP + jj
                                nc.tensor.matmul(
                                    ps[:, jj, i, 0:nw],
                                    pt[:, i, j * 128 + DA[i]:
                                       j * 128 + DA[i] + 128],
                                    wp_t[:, i, 0:nw])
                        if not ABL_NO_B:
                            for i in (1, 2):
                                for jj in range(gsz):
                                    j = g * GRP + jj
                                    nc.tensor.matmul(
                                        ps[:, jj, i, 128:144],
                                        pt[:, i, j * 128 + DB[i]:
                                           j * 128 + DB[i] + 128],
                                        wb_t[:, i, :])
                    else:
                        for jj in range(gsz):
                            j = g * GRP + jj
                            for i in range(3):
                                nw = 144 if i == 0 else 128
                                nc.tensor.matmul(
                                    ps[:, jj, i, 0:nw],
                                    pt[:, i, j * 128 + DA[i]:
                                       j * 128 + DA[i] + 128],
                                    wp_t[:, i, 0:nw])
                                if i != 0 and not ABL_NO_B:
                                    nc.tensor.matmul(
                                        ps[:, jj, i, 128:144],
                                        pt[:, i, j * 128 + DB[i]:
                                           j * 128 + DB[i] + 128],
                                        wb_t[:, i, :])
                    gsl = slice(g * GRP, g * GRP + gsz)
                    if ABL_HALF_SIG:
                        nc.scalar.activation(
                            gates[:, gsl, :, 0:72], ps[:, :, :, 0:72],
                            AF.Sigmoid)
                    else:
                        nc.scalar.activation(
                            gates[:, gsl, :, :], ps[:, :, :, 0:144], AF.Sigmoid)
                return (gates, auxt, tmp, res, blk0, sz)

            def elem_phase(state):
                gates, auxt, tmp, res, blk0, sz = state
                ALL = slice(None)
                g3 = (ALL, ALL, ALL)
                # a = 2*sigmoid(2z) - 1
                V.tensor_scalar(tmp[:, :, :, S5], gates[:, :, :, GA],
                                2.0, -1.0, ALU.mult, ALU.add)
                V.tensor_mul(tmp[:, :, :, S0], gates[:, :, :, GI],
                             tmp[:, :, :, S5])
                E_FC.tensor_mul(tmp[:, :, :, S1], gates[:, :, :, GF],
                                auxt[:, :, :, 0:16])
                V.tensor_add(res[:, :, :, 32:48], tmp[:, :, :, S0],
                             tmp[:, :, :, S1])
                if not ABL_NO_TANH:
                    nc.scalar.activation(tmp[:, :, :, S2],
                                         res[:, :, :, 32:48], AF.Tanh)
                V.tensor_mul(res[:, :, :, 16:32], gates[:, :, :, GO],
                             tmp[:, :, :, S2])
                # r-gate * child products: blocks 0,1 are s-ordered 96:144
                if not ABL_DVE_LITE:
                    V.tensor_mul(tmp[:, :, :, S3], gates[:, :, 0, 96:144],
                                 auxt[:, :, :, 16:32])
                    V.tensor_mul(tmp[:, :, :, S4], gates[:, :, 1, 96:144],
                                 auxt[:, :, :, 32:48])
                    for s in range(3):
                        E_RM2.tensor_mul(
                            tmp[:, :, s, S0], gates[:, :, 2, RCOL[s][2]:
                                                    RCOL[s][2] + 16],
                            auxt[:, :, s, 48:64])
                    V.tensor_add(tmp[:, :, :, S5], tmp[:, :, :, S3],
                                 tmp[:, :, :, S4])
                    V.tensor_add(tmp[:, :, :, S3], tmp[:, :, :, S5],
                                 tmp[:, :, :, S0])
                V.tensor_mul(tmp[:, :, :, S4], gates[:, :, :, GN1],
                             tmp[:, :, :, S3])
                E_N2H.tensor_mul(tmp[:, :, :, S0], gates[:, :, :, GN2],
                                 res[:, :, :, 16:32])
                V.tensor_add(res[:, :, :, 0:16], tmp[:, :, :, S4],
                             tmp[:, :, :, S0])
                # stores ride the Pool queue (loads use SP) to keep the ACT
                # engine free for sigmoids and avoid head-of-line blocking
                res_dst = res_d[blk0 // SZ] if SG_MAJOR \
                    else res_d[:, blk0:blk0 + sz, :, :]
                if ABL_NO_STORE:
                    pass
                elif STORE_GPS:
                    nc.gpsimd.dma_start(res_dst, res[:])
                else:
                    nc.scalar.dma_start(res_dst, res[:])

            def one_pass():
                state = gate_phase(0)
                for sg in range(1, NSG):
                    nstate = gate_phase(sg)
                    elem_phase(state)
                    state = nstate
                elem_phase(state)

            if niter == 1:
                one_pass()
            else:
                kw = {}
                if STAGGER:
                    kw["staggered_reset"] = True
                if HINTS:
                    kw["hint_engines"] = (
                        mybir.EngineType.PE, mybir.EngineType.DVE,
                        mybir.EngineType.Activation, mybir.EngineType.SP,
                        mybir.EngineType.Pool)
                with tc.For_i(0, niter, **kw):
                    one_pass()

    nc.compile()
    return nc


def _get_nc():
    if "nc" not in _NC_CACHE:
        _NC_CACHE["nc"] = build_nc()
    return _NC_CACHE["nc"]


def gather_out(results):
    n = np.empty((B, 16), np.float32)
    h = np.empty((B, 16), np.float32)
    c = np.empty((B, 16), np.float32)
    for m in range(NCORES):
        res = np.asarray(results[m]["res"]).astype(np.float32)
        if SG_MAJOR:
            # [NSG, 128, SZ, 3, 48] -> [128, TPB, 3, 48]
            res = res.transpose(1, 0, 2, 3, 4).reshape(128, TPB, 3, 48)
        # [128, TPB, 3, 48] -> per section s: [TP, 48]
        for s in range(3):
            first = m * R + s
            T = len(range(first, (m + 1) * R, 3))
            flat = res[:, :, s, :].transpose(1, 0, 2).reshape(TP, 48)
            n[first:(m + 1) * R:3] = flat[:T, 0:16]
            h[first:(m + 1) * R:3] = flat[:T, 16:32]
            c[first:(m + 1) * R:3] = flat[:T, 32:48]
    return n, h, c


def make_in_maps(cores):
    return [dict(pall=c["pall"], wp=c["wp"], wb=c["wb"], aux=c["aux"])
            for c in cores]


def kernel(**inputs):
    from concourse.bass_utils import run_bass_kernel_spmd

    cores = host_prep(inputs)
    nc = _get_nc()
    out = run_bass_kernel_spmd(nc, make_in_maps(cores),
                               core_ids=list(range(NCORES)))
    return gather_out(out.results)

